# revision 4
# baseline (speedup 1.0000x reference)
"""Trainium2 Bass kernel for DiagonalLinear.

The reference masks W to its diagonal (zeroing entries with |w| <= 1e-4)
and computes x @ masked_W.T, which is exactly an elementwise scale of
x's columns by the thresholded diagonal of W.

Distribution (8 NeuronCores): data-parallel — x is sharded along the
token axis (1024 tokens per core); per the sharding hint, only the
(thresholded) diagonal of W — 4096 floats, the sole part of W the op
reads — is replicated to every core. No inter-core communication.

The op is purely memory-bound and the f32 version sits exactly at the
per-core DMA roofline (16 MiB in + 16 MiB out). Two levers push below
that roofline:

1. bf16 streaming: x is quantized to bf16 (error <= 2^-8 per rounding,
   and bf16 keeps the full f32 exponent range so the bound holds for
   every element magnitude; three roundings stay under 1.2%, well
   inside the 2e-2 tolerance). Output is stored as bf16 and widened to
   f32 on the host. Halves HBM traffic to 8 MiB in + 8 MiB out.

2. SDMA engine-15 balancing: transfers are split across the 16 SDMA
   engines by SBUF partition (port p serves a fixed set of 8
   partitions; port 15 serves partitions 92-95 and 124-127). Engine 15
   runs ~18% slower than the rest, so with uniform [128, N] tiles it
   finishes ~8 us after every other engine — a serial tail. Main
   tiles here are [124, 4096] (partitions 0-123), which gives port 15
   only 4 lines per tile vs 8 for every other port, so it carries half
   the bytes and is never the straggler. The leftover 32 rows travel
   as one [128, 1024] tile (each row split across 4 partitions), which
   loads every port evenly with 2 KiB lines.

The remainder tile's partitions hold quarter-rows, so its diagonal
pattern is db_rem[p, c] = d[(p%4)*1024 + c]; it is built exactly with
one K=4 matmul of a 0/1 mask [4, 128] against d reshaped to [4, 1024].

Per-core device program — raw Bass (no Tile scheduler) with hand-placed
semaphores, so there are no scheduler-inserted waits and the kernel
ends on a single store-completion wait instead of an all-engine
barrier.

Engine plan (single Block, all engines concurrent):
  sync   : 8 x-tile loads of [124, 4096] bf16 plus the [128, 1024]
           remainder load on the HWDGE qSP ring; once the loads drain
           it also issues the last two stores (tile 7 + remainder)
  tensor : replicate the diagonal across partitions with 8 K=1
           matmuls ones[1,124]^T @ d_row[1,512] -> PSUM (4 banks,
           reused with a copy handshake), then 2 K=4 mask matmuls for
           the remainder pattern
  vector : PSUM->SBUF copies of the replicated diagonal (f32 PSUM
           downcast to bf16 SBUF), one drain, then 9 tile multiplies
  scalar : d-row / d-seg / mask loads plus 7 tile stores on the HWDGE
           qAct ring (separate ring so loads and stores don't
           serialize on one FIFO)
"""

import numpy as np

TOKENS = 8192
N = 4096
N_CORES = 8
T_SHARD = TOKENS // N_CORES  # 1024
P = 124                      # main-tile partitions: keeps port 15 at half load
N_TILES = 8                  # main tiles of 124 rows = 992 rows
REM = T_SHARD - P * N_TILES  # 32 remainder rows
REM_P = 128                  # remainder viewed as [128, 1024]
REM_F = (REM * N) // REM_P   # 1024
MM_N = 512                   # PSUM bank width (fp32)
ACC_BANKS = 4                # PSUM banks cycled by the 8 diag matmuls
THRESHOLD = 1e-4

_CACHED_NC = None


def _build_nc():
    from contextlib import ExitStack

    from concourse import bass, mybir

    bf16 = mybir.dt.bfloat16
    f32 = mybir.dt.float32
    nc = bass.Bass()
    x_in = nc.declare_dram_parameter("x", [T_SHARD, N], bf16, isOutput=False)
    d_in = nc.declare_dram_parameter("d", [N], bf16, isOutput=False)
    mask_in = nc.declare_dram_parameter("mask", [4, REM_P], bf16, isOutput=False)
    out = nc.declare_dram_parameter("out", [T_SHARD, N], bf16, isOutput=True)
    warm = nc.dram_tensor("warm", [2, N], bf16)  # write-path warm-up target

    x_ap = x_in[:]
    o_ap = out[:]
    x_v = x_ap[: P * N_TILES].rearrange("(m p) n -> m p n", p=P)
    o_v = o_ap[: P * N_TILES].rearrange("(m p) n -> m p n", p=P)
    # trailing 32 rows as [128, 1024]: partition 4g+s holds row 992+g,
    # columns s*1024..(s+1)*1024 — a contiguous reshape
    xr_v = x_ap[P * N_TILES :].rearrange("g (s c) -> (g s) c", s=4)
    or_v = o_ap[P * N_TILES :].rearrange("g (s c) -> (g s) c", s=4)
    d_seg_v = d_in[:].rearrange("(q c) -> q c", q=4)

    n_mul = N_TILES + 1

    with ExitStack() as ctx:
        s_ld = [
            ctx.enter_context(nc.semaphore(f"s_ld{i}")) for i in range(n_mul)
        ]
        s_row = ctx.enter_context(nc.semaphore("s_row"))
        s_seg = ctx.enter_context(nc.semaphore("s_seg"))
        s_msk = ctx.enter_context(nc.semaphore("s_msk"))
        s_ones = ctx.enter_context(nc.semaphore("s_ones"))
        s_mm = ctx.enter_context(nc.semaphore("s_mm"))
        s_cp = ctx.enter_context(nc.semaphore("s_cp"))
        s_mul = ctx.enter_context(nc.semaphore("s_mul"))
        s_st = ctx.enter_context(nc.semaphore("s_st"))
        s_st2 = ctx.enter_context(nc.semaphore("s_st2"))
        s_warm = ctx.enter_context(nc.semaphore("s_warm"))

        row = ctx.enter_context(nc.sbuf_tensor("row", [1, N], bf16))
        dseg = ctx.enter_context(nc.sbuf_tensor("dseg", [4, REM_F], bf16))
        mask = ctx.enter_context(nc.sbuf_tensor("mask_sb", [4, REM_P], bf16))
        ones = ctx.enter_context(nc.sbuf_tensor("ones", [1, P], bf16))
        db = ctx.enter_context(nc.sbuf_tensor("db", [P, N], bf16))
        dbr = ctx.enter_context(nc.sbuf_tensor("dbr", [REM_P, REM_F], bf16))
        xts = [
            ctx.enter_context(nc.sbuf_tensor(f"xt{i}", [P, N], bf16))
            for i in range(N_TILES)
        ]
        xr = ctx.enter_context(nc.sbuf_tensor("xr", [REM_P, REM_F], bf16))
        acc = ctx.enter_context(nc.psum_tensor("acc", [P, ACC_BANKS * MM_N], f32))
        acc2 = ctx.enter_context(nc.psum_tensor("acc2", [REM_P, REM_F], f32))

        with nc.Block() as block:

            @block.sync
            def _(sync):
                for i in range(N_TILES):
                    sync.dma_start(out=xts[i][:], in_=x_v[i]).then_inc(s_ld[i], 16)
                sync.dma_start(out=xr[:], in_=xr_v).then_inc(s_ld[N_TILES], 16)
                # last two stores ride the sync ring: it is idle once the
                # loads drain, so the store backlog drains on both rings
                sync.wait_ge(s_ones, 1)
                sync.dma_start(out=warm[0, None, :P], in_=ones[:]).then_inc(
                    s_warm, 16
                )
                sync.wait_ge(s_mul, N_TILES)
                sync.dma_start(out=o_v[N_TILES - 1], in_=xts[N_TILES - 1][:]).then_inc(
                    s_st2, 16
                )
                sync.wait_ge(s_mul, n_mul)
                sync.dma_start(out=or_v, in_=xr[:]).then_inc(s_st2, 16)
                sync.wait_ge(s_st2, 32)
                sync.wait_ge(s_warm, 32)

            @block.tensor
            def _(tensor):
                tensor.wait_ge(s_ones, 1)
                tensor.wait_ge(s_row, 16)
                for j in range(N // MM_N):
                    if j >= ACC_BANKS:
                        tensor.wait_ge(s_cp, j - ACC_BANKS + 1)
                    b = (j % ACC_BANKS) * MM_N
                    tensor.matmul(
                        acc[:, b : b + MM_N],
                        ones[:],
                        row[:, j * MM_N : (j + 1) * MM_N],
                        start=True,
                        stop=True,
                    ).then_inc(s_mm, 1)
                tensor.wait_ge(s_seg, 16)
                tensor.wait_ge(s_msk, 16)
                for j in range(REM_F // MM_N):
                    tensor.matmul(
                        acc2[:, j * MM_N : (j + 1) * MM_N],
                        mask[:],
                        dseg[:, j * MM_N : (j + 1) * MM_N],
                        start=True,
                        stop=True,
                    ).then_inc(s_mm, 1)

            @block.vector
            def _(vector):
                vector.memset(ones[:], 1.0).then_inc(s_ones, 1)
                for j in range(N // MM_N):
                    vector.wait_ge(s_mm, j + 1)
                    b = (j % ACC_BANKS) * MM_N
                    vector.tensor_copy(
                        out=db[:, j * MM_N : (j + 1) * MM_N],
                        in_=acc[:, b : b + MM_N],
                    ).then_inc(s_cp, 1)
                for j in range(REM_F // MM_N):
                    vector.wait_ge(s_mm, N // MM_N + j + 1)
                    vector.tensor_copy(
                        out=dbr[:, j * MM_N : (j + 1) * MM_N],
                        in_=acc2[:, j * MM_N : (j + 1) * MM_N],
                    )
                # DVE writes are pipelined: drain before the muls read
                # db/dbr written by the copies above on this same engine.
                vector.drain()
                for i in range(N_TILES):
                    vector.wait_ge(s_ld[i], 16)
                    vector.tensor_mul(
                        out=xts[i][:], in0=xts[i][:], in1=db[:]
                    ).then_inc(s_mul, 1)
                vector.wait_ge(s_ld[N_TILES], 16)
                vector.tensor_mul(out=xr[:], in0=xr[:], in1=dbr[:]).then_inc(
                    s_mul, 1
                )

            @block.scalar
            def _(scalar):
                # small loads ride the scalar ring: keeps them + their
                # completion receipts off the head of the sync load FIFO
                scalar.dma_start(out=row[:], in_=d_in[None, :]).then_inc(s_row, 16)
                scalar.dma_start(out=dseg[:], in_=d_seg_v).then_inc(s_seg, 16)
                scalar.dma_start(out=mask[:], in_=mask_in[:]).then_inc(s_msk, 16)
                scalar.wait_ge(s_row, 16)
                scalar.dma_start(out=warm[1, None, :], in_=row[:]).then_inc(
                    s_warm, 16
                )
                for i in range(N_TILES - 1):
                    scalar.wait_ge(s_mul, i + 1)
                    scalar.dma_start(out=o_v[i], in_=xts[i][:]).then_inc(s_st, 16)
                scalar.wait_ge(s_st, 16 * (N_TILES - 1))
                scalar.wait_ge(s_warm, 32)

    nc.finalize()
    return nc


def _get_nc():
    global _CACHED_NC
    if _CACHED_NC is None:
        _CACHED_NC = _build_nc()
    return _CACHED_NC


def _shard_inputs(x, W):
    import ml_dtypes

    bf16 = ml_dtypes.bfloat16
    x = np.ascontiguousarray(np.asarray(x, dtype=np.float32)).astype(bf16)
    W = np.asarray(W, dtype=np.float32)
    d = np.ascontiguousarray(np.diagonal(W))
    d = np.where(np.abs(d) > THRESHOLD, d, np.float32(0.0)).astype(bf16)
    mask = np.zeros((4, REM_P), dtype=bf16)
    for k in range(4):
        mask[k, k::4] = bf16(1.0)
    assert x.shape == (TOKENS, N) and d.shape == (N,)
    return [
        {"x": x[c * T_SHARD : (c + 1) * T_SHARD], "d": d, "mask": mask}
        for c in range(N_CORES)
    ]


def _run(x, W, **spmd_kwargs):
    from concourse.bass_utils import run_bass_kernel_spmd

    nc = _get_nc()
    in_maps = _shard_inputs(x, W)
    res = run_bass_kernel_spmd(nc, in_maps, list(range(N_CORES)), **spmd_kwargs)
    out = np.concatenate(
        [res.results[c]["out"] for c in range(N_CORES)], axis=0
    ).astype(np.float32)
    return out, res


def kernel(x, W):
    out, _ = _run(x, W)
    return out


# revision 5
# speedup vs baseline: 2.7732x; 2.7732x over previous
"""Trainium2 Bass kernel for DiagonalLinear.

The reference masks W to its diagonal (zeroing entries with |w| <= 1e-4)
and computes x @ masked_W.T, which is exactly an elementwise scale of
x's columns by the thresholded diagonal of W.

Distribution (8 NeuronCores): data-parallel — x is sharded along the
token axis (1024 tokens per core); per the sharding hint, only the
(thresholded) diagonal of W — 4096 floats, the sole part of W the op
reads — is replicated to every core. No inter-core communication.

The op is purely memory-bound and the f32 version sits exactly at the
per-core DMA roofline (16 MiB in + 16 MiB out). Two levers push below
that roofline:

1. bf16 streaming: x is quantized to bf16 (error <= 2^-8 per rounding,
   and bf16 keeps the full f32 exponent range so the bound holds for
   every element magnitude; three roundings stay under 1.2%, well
   inside the 2e-2 tolerance). Output is stored as bf16 and widened to
   f32 on the host. Halves HBM traffic to 8 MiB in + 8 MiB out.

2. SDMA engine-15 balancing: a DMA's lines are split evenly over the
   first n SDMA engines, where n is the largest divisor of the
   partition count <= 16 (measured on hardware: 128 rows -> 16 engines
   x 8, 120 -> 15 x 8, 112 -> 16 x 7, 124 -> 4 x 31). Engine 15 runs
   ~18% slower than the others, so with uniform [128, N] tiles it
   finishes ~8 us after every other engine — a serial tail on every
   transfer's completion. The shard is instead tiled as 2 x [120, 4096]
   + 7 x [112, 4096] (= 1024 rows): the [120] tiles never touch engine
   15, the [112] tiles load all 16 evenly, so engine 15 carries 0.75x
   the bytes of every other engine and is never the straggler, while
   engines 0-14 stay fully and evenly loaded.

Per-core device program — raw Bass (no Tile scheduler) with hand-placed
semaphores, so there are no scheduler-inserted waits and the kernel
ends on a single store-completion wait instead of an all-engine
barrier.

Engine plan (single Block, all engines concurrent):
  sync   : 9 x-tile loads on the HWDGE qSP ring; once the loads drain
           it also issues the last two stores (tiles 7 and 8)
  tensor : replicate the diagonal across partitions with 8 exact
           K=1 matmuls ones[1,120]^T @ d_row[1,512] -> PSUM banks
           (no extra HBM traffic for the broadcast)
  vector : 8 PSUM->SBUF copies of the replicated diagonal (f32 PSUM
           downcast to bf16 SBUF), one drain, then the 9 tile
           multiplies (the last tile in two halves so its store can
           start earlier)
  scalar : d-row load, then 7 tile stores on the HWDGE qAct ring
           (separate ring so loads and stores don't serialize on one
           FIFO)
"""

import numpy as np

TOKENS = 8192
N = 4096
N_CORES = 8
T_SHARD = TOKENS // N_CORES  # 1024
# 2 x 120-row tiles (15 SDMA engines, skipping slow engine 15) +
# 7 x 112-row tiles (all 16 engines) = 1024 rows
TILE_P = [120, 120, 112, 112, 112, 112, 112, 112, 112]
P0 = TILE_P[0]
MM_N = 512                   # PSUM bank width (fp32)
THRESHOLD = 1e-4

_CACHED_NC = None


def _build_nc():
    from contextlib import ExitStack

    from concourse import bass, mybir

    bf16 = mybir.dt.bfloat16
    f32 = mybir.dt.float32
    nc = bass.Bass()
    x_in = nc.declare_dram_parameter("x", [T_SHARD, N], bf16, isOutput=False)
    d_in = nc.declare_dram_parameter("d", [N], bf16, isOutput=False)
    out = nc.declare_dram_parameter("out", [T_SHARD, N], bf16, isOutput=True)
    warm = nc.dram_tensor("warm", [2, N], bf16)  # write-path warm-up target

    x_ap = x_in[:]
    o_ap = out[:]
    offs = np.cumsum([0] + TILE_P)
    x_v = [x_ap[offs[i] : offs[i + 1]] for i in range(len(TILE_P))]
    o_v = [o_ap[offs[i] : offs[i + 1]] for i in range(len(TILE_P))]

    n_tiles = len(TILE_P)
    n_mul = n_tiles + 1  # last tile multiplied in two halves

    with ExitStack() as ctx:
        s_ld = [
            ctx.enter_context(nc.semaphore(f"s_ld{i}")) for i in range(n_tiles)
        ]
        s_row = ctx.enter_context(nc.semaphore("s_row"))
        s_ones = ctx.enter_context(nc.semaphore("s_ones"))
        s_mm = ctx.enter_context(nc.semaphore("s_mm"))
        s_mul = ctx.enter_context(nc.semaphore("s_mul"))
        s_st = ctx.enter_context(nc.semaphore("s_st"))
        s_st2 = ctx.enter_context(nc.semaphore("s_st2"))
        s_warm = ctx.enter_context(nc.semaphore("s_warm"))

        row = ctx.enter_context(nc.sbuf_tensor("row", [1, N], bf16))
        ones = ctx.enter_context(nc.sbuf_tensor("ones", [1, P0], bf16))
        db = ctx.enter_context(nc.sbuf_tensor("db", [P0, N], bf16))
        xts = [
            ctx.enter_context(nc.sbuf_tensor(f"xt{i}", [p, N], bf16))
            for i, p in enumerate(TILE_P)
        ]
        acc = ctx.enter_context(nc.psum_tensor("acc", [P0, N], f32))

        with nc.Block() as block:

            @block.sync
            def _(sync):
                for i in range(n_tiles):
                    sync.dma_start(out=xts[i][:], in_=x_v[i]).then_inc(s_ld[i], 16)
                # last two stores ride the sync ring: it is idle once the
                # loads drain, so the store backlog drains on both rings
                sync.wait_ge(s_ones, 1)
                sync.dma_start(out=warm[0, None, :P0], in_=ones[:]).then_inc(
                    s_warm, 16
                )
                sync.wait_ge(s_mul, n_tiles - 1)
                sync.dma_start(
                    out=o_v[n_tiles - 2], in_=xts[n_tiles - 2][:]
                ).then_inc(s_st2, 16)
                # tile 8 streams out in two halves as its muls finish
                sync.wait_ge(s_mul, n_tiles)
                sync.dma_start(
                    out=o_v[n_tiles - 1][:, : N // 2],
                    in_=xts[n_tiles - 1][:, : N // 2],
                ).then_inc(s_st2, 16)
                sync.wait_ge(s_mul, n_mul)
                sync.dma_start(
                    out=o_v[n_tiles - 1][:, N // 2 :],
                    in_=xts[n_tiles - 1][:, N // 2 :],
                ).then_inc(s_st2, 16)
                sync.wait_ge(s_st2, 48)
                sync.wait_ge(s_warm, 32)

            @block.tensor
            def _(tensor):
                tensor.wait_ge(s_ones, 1)
                tensor.wait_ge(s_row, 16)
                for j in range(N // MM_N):
                    tensor.matmul(
                        acc[:, j * MM_N : (j + 1) * MM_N],
                        ones[:],
                        row[:, j * MM_N : (j + 1) * MM_N],
                        start=True,
                        stop=True,
                    ).then_inc(s_mm, 1)

            @block.vector
            def _(vector):
                vector.memset(ones[:], 1.0).then_inc(s_ones, 1)
                for j in range(N // MM_N):
                    vector.wait_ge(s_mm, j + 1)
                    vector.tensor_copy(
                        out=db[:, j * MM_N : (j + 1) * MM_N],
                        in_=acc[:, j * MM_N : (j + 1) * MM_N],
                    )
                # DVE writes are pipelined: drain before the muls read db
                # written by the copies above on this same engine.
                vector.drain()
                for i in range(n_tiles - 1):
                    vector.wait_ge(s_ld[i], 16)
                    p = TILE_P[i]
                    vector.tensor_mul(
                        out=xts[i][:], in0=xts[i][:], in1=db[:p, :]
                    ).then_inc(s_mul, 1)
                i = n_tiles - 1
                p = TILE_P[i]
                vector.wait_ge(s_ld[i], 16)
                vector.tensor_mul(
                    out=xts[i][:, : N // 2],
                    in0=xts[i][:, : N // 2],
                    in1=db[:p, : N // 2],
                ).then_inc(s_mul, 1)
                vector.tensor_mul(
                    out=xts[i][:, N // 2 :],
                    in0=xts[i][:, N // 2 :],
                    in1=db[:p, N // 2 :],
                ).then_inc(s_mul, 1)

            @block.scalar
            def _(scalar):
                # d-row load rides the scalar ring: keeps the 8 KiB + its
                # completion receipt off the head of the sync load FIFO
                scalar.dma_start(out=row[:], in_=d_in[None, :]).then_inc(s_row, 16)
                scalar.wait_ge(s_row, 16)
                scalar.dma_start(out=warm[1, None, :], in_=row[:]).then_inc(
                    s_warm, 16
                )
                for i in range(n_tiles - 2):
                    scalar.wait_ge(s_mul, i + 1)
                    scalar.dma_start(out=o_v[i], in_=xts[i][:]).then_inc(s_st, 16)
                scalar.wait_ge(s_st, 16 * (n_tiles - 2))
                scalar.wait_ge(s_warm, 32)

    nc.finalize()
    return nc


def _get_nc():
    global _CACHED_NC
    if _CACHED_NC is None:
        _CACHED_NC = _build_nc()
    return _CACHED_NC


def _shard_inputs(x, W):
    import ml_dtypes

    bf16 = ml_dtypes.bfloat16
    x = np.ascontiguousarray(np.asarray(x, dtype=np.float32)).astype(bf16)
    W = np.asarray(W, dtype=np.float32)
    d = np.ascontiguousarray(np.diagonal(W))
    d = np.where(np.abs(d) > THRESHOLD, d, np.float32(0.0)).astype(bf16)
    assert x.shape == (TOKENS, N) and d.shape == (N,)
    return [
        {"x": x[c * T_SHARD : (c + 1) * T_SHARD], "d": d} for c in range(N_CORES)
    ]


def _run(x, W, **spmd_kwargs):
    from concourse.bass_utils import run_bass_kernel_spmd

    nc = _get_nc()
    in_maps = _shard_inputs(x, W)
    res = run_bass_kernel_spmd(nc, in_maps, list(range(N_CORES)), **spmd_kwargs)
    out = np.concatenate(
        [res.results[c]["out"] for c in range(N_CORES)], axis=0
    ).astype(np.float32)
    return out, res


def kernel(x, W):
    out, _ = _run(x, W)
    return out


# revision 6
# speedup vs baseline: 2.8984x; 1.0451x over previous
"""Trainium2 Bass kernel for DiagonalLinear.

The reference masks W to its diagonal (zeroing entries with |w| <= 1e-4)
and computes x @ masked_W.T, which is exactly an elementwise scale of
x's columns by the thresholded diagonal of W.

Distribution (8 NeuronCores): data-parallel — x is sharded along the
token axis (1024 tokens per core); per the sharding hint, only the
(thresholded) diagonal of W — 4096 floats, the sole part of W the op
reads — is replicated to every core. No inter-core communication.

The op is purely memory-bound and the f32 version sits exactly at the
per-core DMA roofline (16 MiB in + 16 MiB out). Two levers push below
that roofline:

1. bf16 streaming: x is quantized to bf16 (error <= 2^-8 per rounding,
   and bf16 keeps the full f32 exponent range so the bound holds for
   every element magnitude; three roundings stay under 1.2%, well
   inside the 2e-2 tolerance). Output is stored as bf16 and widened to
   f32 on the host. Halves HBM traffic to 8 MiB in + 8 MiB out.

2. SDMA engine-15 balancing: a DMA's lines are split evenly over the
   first n SDMA engines, where n is the largest divisor of the
   partition count <= 16 (measured on hardware: 128 rows -> 16 engines
   x 8, 120 -> 15 x 8, 112 -> 16 x 7, 124 -> 4 x 31). Engine 15 runs
   ~18% slower than the others, so with uniform [128, N] tiles it
   finishes ~8 us after every other engine — a serial tail on every
   transfer's completion. The shard is instead tiled as 2 x [120, 4096]
   + 7 x [112, 4096] (= 1024 rows): the [120] tiles never touch engine
   15, the [112] tiles load all 16 evenly, so engine 15 carries 0.75x
   the bytes of every other engine and is never the straggler, while
   engines 0-14 stay fully and evenly loaded.

Per-core device program — raw Bass (no Tile scheduler) with hand-placed
semaphores, so there are no scheduler-inserted waits and the kernel
ends on a single store-completion wait instead of an all-engine
barrier.

Engine plan (single Block, all engines concurrent):
  sync   : 9 x-tile loads on the HWDGE qSP ring; once the loads drain
           it also issues the last two stores (tiles 7 and 8)
  tensor : replicate the diagonal across partitions with 8 exact
           K=1 matmuls ones[1,120]^T @ d_row[1,512] -> PSUM banks
           (no extra HBM traffic for the broadcast)
  vector : 8 PSUM->SBUF copies of the replicated diagonal (f32 PSUM
           downcast to bf16 SBUF), one drain, then the 9 tile
           multiplies (the last tile in two halves so its store can
           start earlier)
  scalar : d-row load, then 7 tile stores on the HWDGE qAct ring
           (separate ring so loads and stores don't serialize on one
           FIFO)
"""

import numpy as np

TOKENS = 8192
N = 4096
N_CORES = 8
T_SHARD = TOKENS // N_CORES  # 1024
# Tile mix (1024 rows): [128] tiles use the swizzle-aligned fast path
# (full rate even when only one queue is active — the load-only ramp and
# store-only tail), and give engine 15 its 1/16 share. [120] tiles split
# over engines 0-14 only, relieving slow engine 15; they run mid-stream
# where both queues are active (non-128 splits only reach full rate
# when load and store packets interleave). Net: engine 15 carries 0.75x
# the bytes of every other engine, matching its ~0.82x speed.
TILE_P = [128, 128, 120, 120, 16, 128, 128, 128, 128]
P0 = max(TILE_P)
MM_N = 512                   # PSUM bank width (fp32)
THRESHOLD = 1e-4

_CACHED_NC = None


def _build_nc():
    from contextlib import ExitStack

    from concourse import bass, mybir

    bf16 = mybir.dt.bfloat16
    f32 = mybir.dt.float32
    nc = bass.Bass()
    x_in = nc.declare_dram_parameter("x", [T_SHARD, N], bf16, isOutput=False)
    d_in = nc.declare_dram_parameter("d", [N], bf16, isOutput=False)
    out = nc.declare_dram_parameter("out", [T_SHARD, N], bf16, isOutput=True)
    warm = nc.dram_tensor("warm", [2, N], bf16)  # write-path warm-up target

    x_ap = x_in[:]
    o_ap = out[:]
    offs = np.cumsum([0] + TILE_P)
    x_v = [x_ap[offs[i] : offs[i + 1]] for i in range(len(TILE_P))]
    o_v = [o_ap[offs[i] : offs[i + 1]] for i in range(len(TILE_P))]

    n_tiles = len(TILE_P)
    n_mul = n_tiles + 1  # last tile multiplied in two halves

    with ExitStack() as ctx:
        s_ld = [
            ctx.enter_context(nc.semaphore(f"s_ld{i}")) for i in range(n_tiles)
        ]
        s_row = ctx.enter_context(nc.semaphore("s_row"))
        s_ones = ctx.enter_context(nc.semaphore("s_ones"))
        s_mm = ctx.enter_context(nc.semaphore("s_mm"))
        s_mul = ctx.enter_context(nc.semaphore("s_mul"))
        s_st = ctx.enter_context(nc.semaphore("s_st"))
        s_st2 = ctx.enter_context(nc.semaphore("s_st2"))
        s_warm = ctx.enter_context(nc.semaphore("s_warm"))

        row = ctx.enter_context(nc.sbuf_tensor("row", [1, N], bf16))
        ones = ctx.enter_context(nc.sbuf_tensor("ones", [1, P0], bf16))
        db = ctx.enter_context(nc.sbuf_tensor("db", [P0, N], bf16))
        xts = [
            ctx.enter_context(nc.sbuf_tensor(f"xt{i}", [p, N], bf16))
            for i, p in enumerate(TILE_P)
        ]
        acc = ctx.enter_context(nc.psum_tensor("acc", [P0, N], f32))

        with nc.Block() as block:

            @block.sync
            def _(sync):
                for i in range(n_tiles):
                    sync.dma_start(out=xts[i][:], in_=x_v[i]).then_inc(s_ld[i], 16)
                # last two stores ride the sync ring: it is idle once the
                # loads drain, so the store backlog drains on both rings
                sync.wait_ge(s_ones, 1)
                sync.dma_start(out=warm[0, None, :P0], in_=ones[:]).then_inc(
                    s_warm, 16
                )
                sync.wait_ge(s_mul, n_tiles - 1)
                sync.dma_start(
                    out=o_v[n_tiles - 2], in_=xts[n_tiles - 2][:]
                ).then_inc(s_st2, 16)
                # tile 8 streams out in two halves as its muls finish
                sync.wait_ge(s_mul, n_tiles)
                sync.dma_start(
                    out=o_v[n_tiles - 1][:, : N // 2],
                    in_=xts[n_tiles - 1][:, : N // 2],
                ).then_inc(s_st2, 16)
                sync.wait_ge(s_mul, n_mul)
                sync.dma_start(
                    out=o_v[n_tiles - 1][:, N // 2 :],
                    in_=xts[n_tiles - 1][:, N // 2 :],
                ).then_inc(s_st2, 16)
                sync.wait_ge(s_st2, 48)
                sync.wait_ge(s_warm, 32)

            @block.tensor
            def _(tensor):
                tensor.wait_ge(s_ones, 1)
                tensor.wait_ge(s_row, 16)
                for j in range(N // MM_N):
                    tensor.matmul(
                        acc[:, j * MM_N : (j + 1) * MM_N],
                        ones[:],
                        row[:, j * MM_N : (j + 1) * MM_N],
                        start=True,
                        stop=True,
                    ).then_inc(s_mm, 1)

            @block.vector
            def _(vector):
                vector.memset(ones[:], 1.0).then_inc(s_ones, 1)
                for j in range(N // MM_N):
                    vector.wait_ge(s_mm, j + 1)
                    vector.tensor_copy(
                        out=db[:, j * MM_N : (j + 1) * MM_N],
                        in_=acc[:, j * MM_N : (j + 1) * MM_N],
                    )
                # DVE writes are pipelined: drain before the muls read db
                # written by the copies above on this same engine.
                vector.drain()
                for i in range(n_tiles - 1):
                    vector.wait_ge(s_ld[i], 16)
                    p = TILE_P[i]
                    vector.tensor_mul(
                        out=xts[i][:], in0=xts[i][:], in1=db[:p, :]
                    ).then_inc(s_mul, 1)
                i = n_tiles - 1
                p = TILE_P[i]
                vector.wait_ge(s_ld[i], 16)
                vector.tensor_mul(
                    out=xts[i][:, : N // 2],
                    in0=xts[i][:, : N // 2],
                    in1=db[:p, : N // 2],
                ).then_inc(s_mul, 1)
                vector.tensor_mul(
                    out=xts[i][:, N // 2 :],
                    in0=xts[i][:, N // 2 :],
                    in1=db[:p, N // 2 :],
                ).then_inc(s_mul, 1)

            @block.scalar
            def _(scalar):
                # d-row load rides the scalar ring: keeps the 8 KiB + its
                # completion receipt off the head of the sync load FIFO
                scalar.dma_start(out=row[:], in_=d_in[None, :]).then_inc(s_row, 16)
                scalar.wait_ge(s_row, 16)
                scalar.dma_start(out=warm[1, None, :], in_=row[:]).then_inc(
                    s_warm, 16
                )
                for i in range(n_tiles - 2):
                    scalar.wait_ge(s_mul, i + 1)
                    scalar.dma_start(out=o_v[i], in_=xts[i][:]).then_inc(s_st, 16)
                scalar.wait_ge(s_st, 16 * (n_tiles - 2))
                scalar.wait_ge(s_warm, 32)

    nc.finalize()
    return nc


def _get_nc():
    global _CACHED_NC
    if _CACHED_NC is None:
        _CACHED_NC = _build_nc()
    return _CACHED_NC


def _shard_inputs(x, W):
    import ml_dtypes

    bf16 = ml_dtypes.bfloat16
    x = np.ascontiguousarray(np.asarray(x, dtype=np.float32)).astype(bf16)
    W = np.asarray(W, dtype=np.float32)
    d = np.ascontiguousarray(np.diagonal(W))
    d = np.where(np.abs(d) > THRESHOLD, d, np.float32(0.0)).astype(bf16)
    assert x.shape == (TOKENS, N) and d.shape == (N,)
    return [
        {"x": x[c * T_SHARD : (c + 1) * T_SHARD], "d": d} for c in range(N_CORES)
    ]


def _run(x, W, **spmd_kwargs):
    from concourse.bass_utils import run_bass_kernel_spmd

    nc = _get_nc()
    in_maps = _shard_inputs(x, W)
    res = run_bass_kernel_spmd(nc, in_maps, list(range(N_CORES)), **spmd_kwargs)
    out = np.concatenate(
        [res.results[c]["out"] for c in range(N_CORES)], axis=0
    ).astype(np.float32)
    return out, res


def kernel(x, W):
    out, _ = _run(x, W)
    return out


# revision 7
# speedup vs baseline: 2.9373x; 1.0134x over previous
"""Trainium2 Bass kernel for DiagonalLinear.

The reference masks W to its diagonal (zeroing entries with |w| <= 1e-4)
and computes x @ masked_W.T, which is exactly an elementwise scale of
x's columns by the thresholded diagonal of W.

Distribution (8 NeuronCores): data-parallel — x is sharded along the
token axis (1024 tokens per core); per the sharding hint, only the
(thresholded) diagonal of W — 4096 floats, the sole part of W the op
reads — is replicated to every core. No inter-core communication.

The op is purely memory-bound and the f32 version sits exactly at the
per-core DMA roofline (16 MiB in + 16 MiB out). Levers used to push
below that roofline:

1. bf16 streaming: x is quantized to bf16 (error <= 2^-8 per rounding,
   and bf16 keeps the full f32 exponent range so the bound holds for
   every element magnitude; three roundings stay under 1.2%, well
   inside the 2e-2 tolerance). Output is stored as bf16 and widened to
   f32 on the host. Halves HBM traffic to 8 MiB in + 8 MiB out.

2. SDMA engine-15 balancing: a DMA's lines are split evenly over the
   first n SDMA engines, where n is the largest divisor of the
   partition count <= 16 (measured: 128 rows -> 16 engines x 8 lines,
   120 -> 15 x 8, 124 -> 4 x 31). Engine 15 runs ~18% slower than the
   rest, so uniform [128, N] tiles leave an ~8 us engine-15 serial
   tail. Tiling 1024 rows as [128,128,120,120,16,128,128,128,128]
   gives engine 15 0.75x the bytes of every other engine (matching its
   speed) while keeping every other engine fully and evenly loaded.
   [128] tiles also use a port-aligned descriptor layout that runs at
   full rate even when only one queue is active, so they bracket the
   stream (the load-only ramp and store-only tail); the [120]/[16]
   tiles run mid-stream where load and store packets interleave (the
   only regime where their port-crossed layout still hits full rate).

3. Early stores: the diagonal-row load rides at the HEAD of the sync
   load FIFO (its 16 descriptors complete in the first packet round,
   ~1 us, instead of waiting ~7 us behind x-tile packets), the
   PSUM->SBUF broadcast copies run on the scalar/ACT engine (removing
   a ~2 us DVE drain from the critical path), and the first tile's
   multiply is split in halves — so the store stream starts at ~14 us
   instead of ~23 us and load/store packets interleave over the
   [120]-tile window.

Per-core device program — raw Bass (no Tile scheduler) with hand-placed
semaphores, so there are no scheduler-inserted waits and the kernel
ends on a single store-completion wait instead of an all-engine
barrier.

Engine plan (single Block, all engines concurrent):
  sync   : d-row load then 9 x-tile loads on the HWDGE qSP ring; once
           the loads drain it issues the last three stores
  tensor : replicate the diagonal across partitions with 8 exact
           K=1 matmuls ones[1,128]^T @ d_row[1,512] -> PSUM banks
           (no extra HBM traffic for the broadcast)
  vector : the 11 tile multiplies (first and last tiles in halves)
  scalar : 8 PSUM->SBUF copies of the replicated diagonal (f32 PSUM
           downcast to bf16 SBUF), then 8 tile stores on the HWDGE
           qAct ring (separate ring so loads and stores don't
           serialize on one FIFO)
"""

import numpy as np

TOKENS = 8192
N = 4096
N_CORES = 8
T_SHARD = TOKENS // N_CORES  # 1024
TILE_P = [128, 128, 120, 120, 16, 128, 128, 128, 128]
P0 = max(TILE_P)
MM_N = 512                   # PSUM bank width (fp32)
THRESHOLD = 1e-4

_CACHED_NC = None


def _build_nc():
    from contextlib import ExitStack

    from concourse import bass, mybir

    bf16 = mybir.dt.bfloat16
    f32 = mybir.dt.float32
    nc = bass.Bass()
    x_in = nc.declare_dram_parameter("x", [T_SHARD, N], bf16, isOutput=False)
    d_in = nc.declare_dram_parameter("d", [N], bf16, isOutput=False)
    out = nc.declare_dram_parameter("out", [T_SHARD, N], bf16, isOutput=True)
    warm = nc.dram_tensor("warm", [2, N], bf16)  # write-path warm-up target

    x_ap = x_in[:]
    o_ap = out[:]
    offs = np.cumsum([0] + TILE_P)
    x_v = [x_ap[offs[i] : offs[i + 1]] for i in range(len(TILE_P))]
    o_v = [o_ap[offs[i] : offs[i + 1]] for i in range(len(TILE_P))]

    n_tiles = len(TILE_P)
    H = N // 2
    # multiply/store units: tile 0 and the last tile go in two halves
    # (mul_no, store ap, sbuf ap) in issue order
    units = []

    with ExitStack() as ctx:
        s_ld = [
            ctx.enter_context(nc.semaphore(f"s_ld{i}")) for i in range(n_tiles)
        ]
        s_row = ctx.enter_context(nc.semaphore("s_row"))
        s_ones = ctx.enter_context(nc.semaphore("s_ones"))
        s_mm = ctx.enter_context(nc.semaphore("s_mm"))
        s_cp = ctx.enter_context(nc.semaphore("s_cp"))
        s_mul = ctx.enter_context(nc.semaphore("s_mul"))
        s_st = ctx.enter_context(nc.semaphore("s_st"))
        s_st2 = ctx.enter_context(nc.semaphore("s_st2"))
        s_warm = ctx.enter_context(nc.semaphore("s_warm"))

        row = ctx.enter_context(nc.sbuf_tensor("row", [1, N], bf16))
        ones = ctx.enter_context(nc.sbuf_tensor("ones", [1, P0], bf16))
        db = ctx.enter_context(nc.sbuf_tensor("db", [P0, N], bf16))
        xts = [
            ctx.enter_context(nc.sbuf_tensor(f"xt{i}", [p, N], bf16))
            for i, p in enumerate(TILE_P)
        ]
        acc = ctx.enter_context(nc.psum_tensor("acc", [P0, N], f32))

        # (tile, col_slice) units in mul order
        last = n_tiles - 1
        units = [(0, slice(0, H)), (0, slice(H, N))]
        units += [(i, slice(0, N)) for i in range(1, last)]
        units += [(last, slice(0, H)), (last, slice(H, N))]
        n_mul = len(units)          # 11
        n_scalar_units = 8          # tile0 halves + tiles 1..6
        n_sync_units = n_mul - n_scalar_units  # tile 7, tile 8 halves

        with nc.Block() as block:

            @block.sync
            def _(sync):
                # d-row load heads the load FIFO: its 16 descriptors are
                # the first packet every engine drains (~1 us) instead of
                # queueing behind x-tile packets on the other ring
                sync.dma_start(out=row[:], in_=d_in[None, :]).then_inc(s_row, 16)
                for i in range(n_tiles):
                    sync.dma_start(out=xts[i][:], in_=x_v[i]).then_inc(s_ld[i], 16)
                sync.wait_ge(s_row, 16)
                sync.dma_start(out=warm[0, None, :], in_=row[:]).then_inc(
                    s_warm, 16
                )
                # last three stores ride the sync ring: it is idle once
                # the loads drain, so the store backlog drains on both rings
                for k in range(n_scalar_units, n_mul):
                    t, cs = units[k]
                    sync.wait_ge(s_mul, k + 1)
                    sync.dma_start(out=o_v[t][:, cs], in_=xts[t][:, cs]).then_inc(
                        s_st2, 16
                    )
                sync.wait_ge(s_st2, 16 * n_sync_units)
                sync.wait_ge(s_warm, 32)

            @block.tensor
            def _(tensor):
                tensor.wait_ge(s_ones, 1)
                tensor.wait_ge(s_row, 16)
                for j in range(N // MM_N):
                    tensor.matmul(
                        acc[:, j * MM_N : (j + 1) * MM_N],
                        ones[:],
                        row[:, j * MM_N : (j + 1) * MM_N],
                        start=True,
                        stop=True,
                    ).then_inc(s_mm, 1)

            @block.vector
            def _(vector):
                vector.memset(ones[:], 1.0).then_inc(s_ones, 1)
                vector.wait_ge(s_cp, N // MM_N)
                for k, (t, cs) in enumerate(units):
                    p = TILE_P[t]
                    if k == 0 or cs.start == 0:
                        vector.wait_ge(s_ld[t], 16)
                    vector.tensor_mul(
                        out=xts[t][:, cs], in0=xts[t][:, cs], in1=db[:p, cs]
                    ).then_inc(s_mul, 1)

            @block.scalar
            def _(scalar):
                scalar.wait_ge(s_row, 16)
                scalar.dma_start(out=warm[1, None, :], in_=row[:]).then_inc(
                    s_warm, 16
                )
                # PSUM -> SBUF broadcast copies on ACT: keeps the DVE free
                # of a drain between writer and reader on the same engine
                for j in range(N // MM_N):
                    scalar.wait_ge(s_mm, j + 1)
                    scalar.copy(
                        out=db[:, j * MM_N : (j + 1) * MM_N],
                        in_=acc[:, j * MM_N : (j + 1) * MM_N],
                    ).then_inc(s_cp, 1)
                for k in range(n_scalar_units):
                    t, cs = units[k]
                    scalar.wait_ge(s_mul, k + 1)
                    scalar.dma_start(
                        out=o_v[t][:, cs], in_=xts[t][:, cs]
                    ).then_inc(s_st, 16)
                scalar.wait_ge(s_st, 16 * n_scalar_units)
                scalar.wait_ge(s_warm, 32)

    nc.finalize()
    return nc


def _get_nc():
    global _CACHED_NC
    if _CACHED_NC is None:
        _CACHED_NC = _build_nc()
    return _CACHED_NC


def _shard_inputs(x, W):
    import ml_dtypes

    bf16 = ml_dtypes.bfloat16
    x = np.ascontiguousarray(np.asarray(x, dtype=np.float32)).astype(bf16)
    W = np.asarray(W, dtype=np.float32)
    d = np.ascontiguousarray(np.diagonal(W))
    d = np.where(np.abs(d) > THRESHOLD, d, np.float32(0.0)).astype(bf16)
    assert x.shape == (TOKENS, N) and d.shape == (N,)
    return [
        {"x": x[c * T_SHARD : (c + 1) * T_SHARD], "d": d} for c in range(N_CORES)
    ]


def _run(x, W, **spmd_kwargs):
    from concourse.bass_utils import run_bass_kernel_spmd

    nc = _get_nc()
    in_maps = _shard_inputs(x, W)
    res = run_bass_kernel_spmd(nc, in_maps, list(range(N_CORES)), **spmd_kwargs)
    out = np.concatenate(
        [res.results[c]["out"] for c in range(N_CORES)], axis=0
    ).astype(np.float32)
    return out, res


def kernel(x, W):
    out, _ = _run(x, W)
    return out
